# revision 1
# baseline (speedup 1.0000x reference)
"""Chamfer loss (B=4, N=M=8192, D=3) on 8 NeuronCores via Bass/Tile.

Strategy:
  - Shard: 8 cores = 4 batches x 2 halves of the gts (i) axis.
    Each core computes P[i, j] = ||gts_i - preds_j||^2 for its 4096 i's
    against all 8192 j's, flash-style (P never materialized in DRAM).
  - P tiles come straight out of one matmul via the augmented contraction
      P[i,j] = sum_k A[i,k] * B[j,k],  A = [-2*gts, 1, ||gts||^2],
                                       B = [preds, ||preds||^2, 1]
    with each factor split hi/lo into fp16 (K = 2*2*5 = 20) so the PE runs
    at full (1 cycle/row) rate while keeping ~2^-22 relative accuracy.
  - ScalarE casts each PSUM tile to fp16 in SBUF; VectorE (the bottleneck)
    runs fp16 2x-mode running minima: an elementwise-min column accumulator
    [128, 8192] and a wide row accumulator W reduced once per i-tile.
  - Column finale: transpose the column accumulator through the PE against
    an identity matrix, then 3D-AP min-reduce -> colmin partials [128, 64].
  - Host: sum row minima; elementwise-min colmin partials across the two
    i-half cores of each batch, then sum.
"""

import numpy as np
from contextlib import ExitStack

from concourse import bass, bacc, mybir
from concourse import tile
from concourse.bass_utils import run_bass_kernel_spmd

B, N, M, D = 4, 8192, 8192, 3
N_CORES = 8
N_I = N // 2          # i rows per core
CH = 2048             # j chunk width (4 PSUM banks)
KDIM = 20             # augmented contraction after fp16 hi/lo split
F16 = mybir.dt.float16
F32 = mybir.dt.float32
BIG = 60000.0         # > max possible P (~120), < fp16 max


def build_nc(n_i=N_I, m=M, ch=CH, trace_sim=False, repeat=1,
             do_row=True, do_col=True, do_act=True,
             wide_act=True, halve_reduce=True, t0_direct=True, gps_col=False,
             big_s=False, ttr_row=False, deep_bufs=False, packed_pe=False,
             n1024=False):
    """Build the per-core Bass program (same NEFF on all 8 cores).

    repeat: emit the main loop `repeat` times (min is idempotent, results
    unchanged) — used to measure marginal HW exec time per pass.
    do_row/do_col/do_act: ablation switches (wrong results when False).
    """
    NT = n_i // 128       # i-tiles
    NCH = m // ch         # chunks per i-tile
    NMM = ch // 512       # matmuls per chunk
    NBLK = m // 128       # col-finale transpose blocks
    GRP = min(16, NBLK)   # blocks per finale group ([128, GRP*128] f32 PSUM)
    NGRP = NBLK // GRP
    assert n_i % 128 == 0 and m % ch == 0 and ch % 512 == 0 and NBLK % GRP == 0

    nc = bacc.Bacc("TRN2", target_bir_lowering=False, debug=False)
    kp = 128 if packed_pe else KDIM
    lhsT_d = nc.dram_tensor("lhsT", [kp, n_i], F16, kind="ExternalInput").ap()
    rhs_d = nc.dram_tensor("rhs", [kp, m], F16, kind="ExternalInput").ap()
    ident_d = nc.dram_tensor("ident", [128, 128], F16, kind="ExternalInput").ap()
    rowmin_d = nc.dram_tensor("rowmin", [128, NT], F32, kind="ExternalOutput").ap()
    colmin_d = nc.dram_tensor("colmin", [128, NBLK], F32, kind="ExternalOutput").ap()

    mn = mybir.AluOpType.min

    with tile.TileContext(nc, trace_sim=trace_sim) as tc, ExitStack() as ctx:
        singles = ctx.enter_context(tc.tile_pool(name="singles", bufs=1))
        spool = ctx.enter_context(tc.tile_pool(name="spool", bufs=3))
        wpool = ctx.enter_context(tc.tile_pool(name="wpool", bufs=2))
        pspool = ctx.enter_context(tc.tile_pool(name="pspool", bufs=2, space="PSUM"))

        lhsT_sb = singles.tile([kp, n_i], F16)
        nc.sync.dma_start(out=lhsT_sb, in_=lhsT_d)
        rhs_sb = singles.tile([kp, m], F16)
        nc.sync.dma_start(out=rhs_sb, in_=rhs_d)
        ident_sb = singles.tile([128, 128], F16)
        nc.sync.dma_start(out=ident_sb, in_=ident_d)

        colacc = singles.tile([128, m], F16)
        if not t0_direct:
            nc.gpsimd.memset(colacc, BIG)
        rowacc = singles.tile([128, NT], F32)
        colfin = singles.tile([128, NBLK], F32)
        if not do_row:
            nc.vector.memset(rowacc, 0.0)

        if ttr_row:
            # Row path via tensor_tensor_reduce over chunk PAIRS of one
            # i-tile: out = min(S_even, S_odd) (discarded scratch) while the
            # fused min-reduce with chained initial value lands the row min
            # directly in rowacc — the whole reduce chain disappears.
            assert NCH % 2 == 0
            rowacc_b = singles.tile([128, NT], F32)
            if NCH < 4:
                nc.vector.memset(rowacc_b, BIG)
            first_pass = True
            for t in [tt for _ in range(repeat) for tt in range(NT)]:
                t0d = t0_direct and t == 0 and first_pass
                first_pass = False
                schunks = []
                for c in range(NCH):
                    if t0d:
                        s_cur = colacc[:, ch * c:ch * (c + 1)]
                    else:
                        s = spool.tile([128, ch], F16, tag="s", bufs=5)
                        s_cur = s
                    ps = pspool.tile([128, ch], F32, tag="ps", bufs=2)
                    for q in range(NMM):
                        nc.tensor.matmul(
                            ps[:, 512 * q:512 * (q + 1)],
                            lhsT_sb[:, 128 * t:128 * (t + 1)],
                            rhs_sb[:, ch * c + 512 * q: ch * c + 512 * (q + 1)],
                            start=True, stop=True,
                        )
                    nc.scalar.copy(out=s_cur, in_=ps)
                    schunks.append(s_cur)
                    if not t0d:
                        cs = colacc[:, ch * c:ch * (c + 1)]
                        nc.vector.tensor_tensor(out=cs, in0=cs, in1=s_cur, op=mn)
                    if c % 2 == 1:
                        wsc = wpool.tile([128, ch], F16, tag="w")
                        acc = rowacc if c == 1 else rowacc_b
                        nc.vector.tensor_tensor_reduce(
                            out=wsc, in0=schunks[c - 1], in1=s_cur, scale=1.0,
                            scalar=BIG, op0=mn, op1=mn,
                            accum_out=acc[:, t:t + 1],
                        )
            # combine the two per-i-tile partial row minima (tiny)
            nc.vector.tensor_tensor(out=rowacc, in0=rowacc, in1=rowacc_b, op=mn)

        elif big_s:
            # fused variant: one [128, m] S buffer per i-tile -> single wide
            # col TT + pure log-halving row chain (fewer DVE instructions)
            first_pass = True
            for t in [tt for _ in range(repeat) for tt in range(NT)]:
                t0d = t0_direct and t == 0 and first_pass
                first_pass = False
                if t0d:
                    sfull = colacc
                else:
                    sfull = spool.tile([128, m], F16, tag="sbig", bufs=2)
                for c in range(NCH):
                    ps = pspool.tile([128, ch], F32, tag="ps", bufs=2)
                    for q in range(NMM):
                        nc.tensor.matmul(
                            ps[:, 512 * q:512 * (q + 1)],
                            lhsT_sb[:, 128 * t:128 * (t + 1)],
                            rhs_sb[:, ch * c + 512 * q: ch * c + 512 * (q + 1)],
                            start=True, stop=True,
                        )
                    nc.scalar.copy(out=sfull[:, ch * c:ch * (c + 1)], in_=ps)
                if not t0d:
                    nc.vector.tensor_tensor(
                        out=colacc, in0=colacc, in1=sfull, op=mn)
                cur, width, hidx = sfull, m, 0
                while width > 512:
                    nxt = wpool.tile([128, width // 2], F16, tag=f"h{hidx}")
                    nc.vector.tensor_tensor(
                        out=nxt, in0=cur[:, :width // 2],
                        in1=cur[:, width // 2:], op=mn)
                    cur, width, hidx = nxt, width // 2, hidx + 1
                nc.vector.tensor_reduce(
                    out=rowacc[:, t:t + 1], in_=cur,
                    axis=mybir.AxisListType.X, op=mn)

        first_pass = True
        for t in ([] if (big_s or ttr_row) else
                  [tt for _ in range(repeat) for tt in range(NT)]):
            t0d = t0_direct and t == 0 and first_pass
            w = wpool.tile([128, ch], F16, tag="w",
                           bufs=3 if deep_bufs else None)
            for c in range(NCH):
                if t0d:
                    # i-tile 0: ScalarE writes colacc directly; no col TT needed
                    s_cur = colacc[:, ch * c:ch * (c + 1)]
                elif c == 0:
                    s_cur = w  # first chunk: ScalarE writes the row accum directly
                else:
                    s = spool.tile([128, ch], F16, tag="s",
                                   bufs=5 if deep_bufs else None)
                    s_cur = s
                if wide_act and n1024:
                    # halve PE instruction count: fp16 moving operand max is
                    # 1024, each MM spans 2 PSUM banks
                    ps = pspool.tile([128, ch], F32, tag="ps", bufs=2)
                    for q in range(ch // 1024):
                        nc.tensor.matmul(
                            ps[:, 1024 * q:1024 * (q + 1)],
                            lhsT_sb[:, 128 * t:128 * (t + 1)],
                            rhs_sb[:, ch * c + 1024 * q: ch * c + 1024 * (q + 1)],
                            start=True, stop=True,
                        )
                    if do_act:
                        nc.scalar.copy(out=s_cur, in_=ps)
                elif wide_act:
                    ps = pspool.tile([128, ch], F32, tag="ps", bufs=2)
                    for q in range(NMM):
                        if packed_pe:
                            bp = 32 * (q % 4)
                            nc.tensor.matmul(
                                ps[:, 512 * q:512 * (q + 1)],
                                lhsT_sb[bp:bp + KDIM, 128 * t:128 * (t + 1)],
                                rhs_sb[bp:bp + KDIM,
                                       ch * c + 512 * q: ch * c + 512 * (q + 1)],
                                start=True, stop=True,
                                tile_position=(bp, 0),
                            )
                        else:
                            nc.tensor.matmul(
                                ps[:, 512 * q:512 * (q + 1)],
                                lhsT_sb[:, 128 * t:128 * (t + 1)],
                                rhs_sb[:, ch * c + 512 * q: ch * c + 512 * (q + 1)],
                                start=True, stop=True,
                            )
                    if do_act:
                        nc.scalar.copy(out=s_cur, in_=ps)
                else:
                    for q in range(NMM):
                        ps = pspool.tile([128, 512], F32, tag="ps", bufs=4)
                        nc.tensor.matmul(
                            ps,
                            lhsT_sb[:, 128 * t:128 * (t + 1)],
                            rhs_sb[:, ch * c + 512 * q: ch * c + 512 * (q + 1)],
                            start=True, stop=True,
                        )
                        if do_act:
                            nc.scalar.copy(out=s_cur[:, 512 * q:512 * (q + 1)], in_=ps)
                if t0d and do_row:
                    # row accum for i-tile 0 reads the colacc slices
                    if c == 0:
                        pass  # w seeded at c == 1 from colacc chunk 0
                    elif c == 1:
                        nc.vector.tensor_tensor(
                            out=w, in0=colacc[:, 0:ch], in1=s_cur, op=mn)
                    else:
                        nc.vector.tensor_tensor(out=w, in0=w, in1=s_cur, op=mn)
                elif do_row and c > 0:
                    nc.vector.tensor_tensor(out=w, in0=w, in1=s_cur, op=mn)
                if do_col and not t0d:
                    # column path: running elementwise min
                    cs = colacc[:, ch * c:ch * (c + 1)]
                    eng = nc.gpsimd if (gps_col and c == NCH - 1) else nc.vector
                    eng.tensor_tensor(out=cs, in0=cs, in1=s_cur, op=mn)
            first_pass = False
            if do_row:
                if halve_reduce and ch >= 2048:
                    h1 = wpool.tile([128, ch // 2], F16, tag="h1")
                    nc.vector.tensor_tensor(
                        out=h1, in0=w[:, :ch // 2], in1=w[:, ch // 2:], op=mn)
                    h2 = wpool.tile([128, ch // 4], F16, tag="h2")
                    nc.vector.tensor_tensor(
                        out=h2, in0=h1[:, :ch // 4], in1=h1[:, ch // 4:], op=mn)
                    nc.vector.tensor_reduce(
                        out=rowacc[:, t:t + 1], in_=h2,
                        axis=mybir.AxisListType.X, op=mn)
                else:
                    nc.vector.tensor_reduce(
                        out=rowacc[:, t:t + 1], in_=w,
                        axis=mybir.AxisListType.X, op=mn)

        # column finale: partition-axis min via PE transpose + free-axis reduce
        for g in range(NGRP):
            if wide_act and GRP * 128 == ch:
                pst = pspool.tile([128, GRP * 128], F32, tag="ps", bufs=2)
            else:
                pst = pspool.tile([128, GRP * 128], F32, tag="psfin", bufs=1)
            for k in range(GRP):
                blk = g * GRP + k
                nc.tensor.matmul(
                    pst[:, 128 * k:128 * (k + 1)],
                    colacc[:, 128 * blk:128 * (blk + 1)],
                    ident_sb,
                    start=True, stop=True,
                )
            nc.vector.tensor_reduce(
                out=colfin[:, g * GRP:(g + 1) * GRP],
                in_=pst.rearrange("p (b x) -> p b x", x=128),
                axis=mybir.AxisListType.X, op=mn,
            )

        nc.sync.dma_start(out=rowmin_d, in_=rowacc)
        nc.sync.dma_start(out=colmin_d, in_=colfin)
    nc.compile()
    return nc


def _split16(x):
    hi = x.astype(np.float16)
    lo = (x - hi.astype(np.float32)).astype(np.float16)
    return hi, lo


def prep_core_inputs(gts_b, preds_b):
    """Augmented, fp16 hi/lo split operands for one core.

    gts_b: [n_i, 3] f32 (this core's i rows), preds_b: [m, 3] f32.
    Returns lhsT [20, n_i] f16, rhs [20, m] f16.
    """
    gts_b = np.asarray(gts_b, dtype=np.float32)
    preds_b = np.asarray(preds_b, dtype=np.float32)
    xx = np.sum(gts_b * gts_b, axis=1, dtype=np.float32)
    yy = np.sum(preds_b * preds_b, axis=1, dtype=np.float32)
    ones_a = np.ones((gts_b.shape[0],), np.float32)
    ones_b = np.ones((preds_b.shape[0],), np.float32)
    A = np.concatenate([-2.0 * gts_b, ones_a[:, None], xx[:, None]], axis=1)  # [n,5]
    Bm = np.concatenate([preds_b, yy[:, None], ones_b[:, None]], axis=1)      # [m,5]
    Ah, Al = _split16(A)
    Bh, Bl = _split16(Bm)
    lhsT = np.concatenate([Ah, Ah, Al, Al], axis=1).T.copy()  # [20, n]
    rhs = np.concatenate([Bh, Bl, Bh, Bl], axis=1).T.copy()   # [20, m]
    return lhsT, rhs


def prep_core_inputs_packed(gts_b, preds_b):
    """prep_core_inputs replicated at partition offsets 0/32/64/96 for
    tile_position row-group packing."""
    lhsT, rhs = prep_core_inputs(gts_b, preds_b)
    Lp = np.zeros((128, lhsT.shape[1]), np.float16)
    Rp = np.zeros((128, rhs.shape[1]), np.float16)
    for g in range(4):
        Lp[32 * g:32 * g + KDIM] = lhsT
        Rp[32 * g:32 * g + KDIM] = rhs
    return Lp, Rp


def combine_outputs(results, m=M):
    """results: list of 8 dicts with 'rowmin' [128, NT] and 'colmin' [128, NBLK]."""
    total = 0.0
    for b in range(len(results) // 2):
        r0, r1 = results[2 * b], results[2 * b + 1]
        total += np.sum(r0["rowmin"], dtype=np.float64)
        total += np.sum(r1["rowmin"], dtype=np.float64)
        c0 = r0["colmin"].T.reshape(-1)  # colfin[p, blk] -> j = blk*128 + p
        c1 = r1["colmin"].T.reshape(-1)
        total += np.sum(np.minimum(c0, c1), dtype=np.float64)
    return np.float32(total)


_NC_CACHE = {}


def kernel(gts, preds, _trace=False):
    gts = np.asarray(gts, dtype=np.float32)
    preds = np.asarray(preds, dtype=np.float32)
    assert gts.shape == (B, N, D) and preds.shape == (B, M, D)

    key = (N_I, M, CH)
    if key not in _NC_CACHE:
        _NC_CACHE[key] = build_nc()
    nc = _NC_CACHE[key]

    ident = np.eye(128, dtype=np.float16)
    in_maps = []
    for c in range(N_CORES):
        b, half = c // 2, c % 2
        lhsT, rhs = prep_core_inputs(
            gts[b, half * N_I:(half + 1) * N_I], preds[b]
        )
        in_maps.append({"lhsT": lhsT, "rhs": rhs, "ident": ident})

    res = run_bass_kernel_spmd(nc, in_maps, list(range(N_CORES)), trace=_trace)
    out = combine_outputs(res.results)
    if _trace:
        return out, res
    return out

